# revision 36
# baseline (speedup 1.0000x reference)
"""Two-layer GCN (PointRefinerGNN) on 8 trn2 cores.

Math: An = norm(adj+I); h = relu(An @ (x@W1)); y = An @ (h@W2); out = x + alpha*y
Sharding: core c owns rows R_c = [c*1024, (c+1)*1024).
Host ships p-major-permuted adjT (fp8 {0,1}), xT/W bf16, x f32 residual,
and precomputed dinv = rsqrt(rowsum+1) transposed tiles.
Device: V1T = (x@W1).T, scale by dinv, AllGather (fp8), aggregate
P1T = feat.T @ adjT with fp8 DoubleRow matmuls, fused relu(dinv^2*P1),
AllGather, aggregate again, Z = Q2@W2, out = x + alpha*dinv*Z.
"""

import numpy as np

N = 8192
IN_DIM = 1024
HID = 256
CORES = 8
R = N // CORES          # 1024 rows per core
NB = R // 128           # 8 row-blocks per core
KB = IN_DIM // 128      # 8 contraction blocks for x@W1
JB = N // 128           # 64 j-blocks for aggregation

LAST_RESULTS = None
USE_FP8_FEAT = True     # featg + AllGather buffers in fp8e4 (else bf16)
USE_DR = True           # DoubleRow perf mode for aggregation matmuls
XRES_PRELOAD = True     # preload all of xres to SBUF (else per-block loads)
AGIN_BATCH = True       # single batched p-major agin write (else per-block)
CHUNK_LOADS = True      # chunked xT/adjT input DMAs, W2 last
RING_REORDER = True     # bulk loads via scalar ring after agin1 write


def _build_program(alpha: float):
    import concourse.bacc as bacc
    import concourse.tile as tile
    import concourse.mybir as mybir
    from concourse import masks

    f32 = mybir.dt.float32
    bf16 = mybir.dt.bfloat16
    fp8 = mybir.dt.float8e4
    featdt = fp8 if USE_FP8_FEAT else bf16
    AF = mybir.ActivationFunctionType
    DR = mybir.MatmulPerfMode.DoubleRow

    nc = bacc.Bacc(
        "TRN2",
        target_bir_lowering=False,
        debug=False,
        enable_asserts=True,
        num_devices=CORES,
    )

    # adjT rows permuted p-major on host: dram row p*JB + n holds source node
    # j = n*128 + p, so each SBUF partition reads one contiguous 64KB run.
    adjT = nc.dram_tensor("adjT", [N, R], fp8, kind="ExternalInput")
    xT = nc.dram_tensor("xT", [IN_DIM, R], bf16, kind="ExternalInput")   # p-major
    xres = nc.dram_tensor("xres", [R, IN_DIM], f32, kind="ExternalInput")
    W1d = nc.dram_tensor("W1", [IN_DIM, HID], bf16, kind="ExternalInput")  # p-major
    W2d = nc.dram_tensor("W2", [HID, IN_DIM], bf16, kind="ExternalInput")  # p-major
    dinvTd = nc.dram_tensor("dinvT", [128, NB], f32, kind="ExternalInput")
    dinv2Td = nc.dram_tensor("dinv2T", [128, NB], f32, kind="ExternalInput")
    adinvTd = nc.dram_tensor("adinvT", [128, NB], f32, kind="ExternalInput")
    out = nc.dram_tensor("out", [R, IN_DIM], f32, kind="ExternalOutput")

    groups = [list(range(CORES))]

    with tile.TileContext(nc) as tc:
        with (
            tc.tile_pool(name="const", bufs=1) as const,
            tc.tile_pool(name="stage", bufs=2) as stage,
            tc.tile_pool(name="big", bufs=3, space="PSUM") as big,
            tc.tile_pool(name="small", bufs=2, space="PSUM") as small,
            tc.tile_pool(name="dram", bufs=1, space="DRAM") as dram,
        ):
            id_bf16 = const.tile([128, 128], bf16)
            masks.make_identity(nc, id_bf16)

            dinvT_sb = const.tile([128, NB], f32)
            nc.sync.dma_start(dinvT_sb[:], dinvTd[:])
            dinv2T_sb = const.tile([128, NB], f32)
            nc.sync.dma_start(dinv2T_sb[:], dinv2Td[:])
            adinvT_sb = const.tile([128, NB], f32)
            nc.sync.dma_start(adinvT_sb[:], adinvTd[:])

            W1_sb = const.tile([128, KB, HID], bf16)
            xT_sb = const.tile([128, KB, R], bf16)
            adjT_sb = const.tile([128, JB, R], fp8)
            W2_sb = const.tile([128, HID // 128, IN_DIM], bf16)
            xT_re = xT[:].rearrange("(p n) f -> p n f", p=128)
            adjT_re = adjT[:].rearrange("(p n) i -> p n i", p=128)

            if RING_REORDER:
                # sync ring carries only what V1T needs; bulk streams go on
                # the scalar ring after the agin1 write (arrival-order FIFOs).
                nc.sync.dma_start(W1_sb[:], W1d[:].rearrange("(p n) f -> p n f", p=128))
                for ck in range(4):
                    nc.sync.dma_start(
                        xT_sb[:, ck * 2:(ck + 1) * 2, :],
                        xT_re[:, ck * 2:(ck + 1) * 2, :],
                    )
            elif CHUNK_LOADS:
                nc.sync.dma_start(W1_sb[:], W1d[:].rearrange("(p n) f -> p n f", p=128))
                for ck in range(4):
                    nc.sync.dma_start(
                        xT_sb[:, ck * 2:(ck + 1) * 2, :],
                        xT_re[:, ck * 2:(ck + 1) * 2, :],
                    )
                for ck in range(16):
                    nc.sync.dma_start(
                        adjT_sb[:, ck * 4:(ck + 1) * 4, :],
                        adjT_re[:, ck * 4:(ck + 1) * 4, :],
                    )
                nc.sync.dma_start(W2_sb[:], W2d[:].rearrange("(p n) f -> p n f", p=128))
            else:
                nc.sync.dma_start(W1_sb[:], W1d[:].rearrange("(p n) f -> p n f", p=128))
                nc.sync.dma_start(W2_sb[:], W2d[:].rearrange("(p n) f -> p n f", p=128))
                nc.sync.dma_start(xT_sb[:], xT_re[:, :, :])
                for ck in range(8):
                    nc.sync.dma_start(
                        adjT_sb[:, ck * 8:(ck + 1) * 8, :],
                        adjT_re[:, ck * 8:(ck + 1) * 8, :],
                    )

            if XRES_PRELOAD:
                xres_sb = const.tile([128, NB, IN_DIM], f32)
                if not RING_REORDER:
                    nc.sync.dma_start(
                        xres_sb[:], xres[:].rearrange("(p n) f -> p n f", p=128)
                    )

            featg = const.tile([128, JB, HID], featdt)

            V1T_sb = const.tile([128, 2, R], bf16)
            P1T_sb = const.tile([128, 2, R], bf16)
            G2T_sb = const.tile([128, 2, R], bf16)

            # AllGather bounce buffers, fp8, p-major rows (row p*NB+b = node
            # b*128+p of the contributing core).
            agin1 = dram.tile([R, HID], featdt)
            agout1 = dram.tile([N, HID], featdt, addr_space="Shared")
            agin2 = dram.tile([R, HID], featdt)
            agout2 = dram.tile([N, HID], featdt, addr_space="Shared")
            agin1_re = agin1.rearrange("(p n) f -> p n f", p=128)
            agin2_re = agin2.rearrange("(p n) f -> p n f", p=128)
            # agout row = c*R + p*NB + b  ->  featg[p, jb=c*NB+b, f]
            agout1_re = agout1.rearrange("(c p n) f -> p c n f", c=CORES, p=128)
            agout2_re = agout2.rearrange("(c p n) f -> p c n f", c=CORES, p=128)

            # V1T = (x_c @ W1).T  [256, 1024]
            v1t = [big.tile([128, R], f32, tag="big", name=f"v1t{i}") for i in range(2)]
            for kb in range(KB):
                for fc in range(2):
                    for h in range(2):
                        nc.tensor.matmul(
                            v1t[fc][:, h * 512:(h + 1) * 512],
                            W1_sb[:, kb, fc * 128:(fc + 1) * 128],
                            xT_sb[:, kb, h * 512:(h + 1) * 512],
                            start=(kb == 0),
                            stop=(kb == KB - 1),
                        )
            for fc in range(2):
                nc.vector.tensor_copy(V1T_sb[:, fc, :], v1t[fc][:])

            # epilogue-1: agin1[i,:] = dinv_i * V1[i,:]  (fp8)
            def epilogue(src_sb, agin_re, act, scale_sb, nm):
                if AGIN_BATCH:
                    stg_all = const.tile([128, NB, HID], featdt, name=f"stg{nm}")
                    for b in range(NB):
                        ps = small.tile([128, HID], bf16, tag="small")
                        for fc in range(2):
                            nc.tensor.transpose(
                                ps[:, fc * 128:(fc + 1) * 128],
                                src_sb[:, fc, b * 128:(b + 1) * 128],
                                id_bf16[:],
                            )
                        nc.scalar.activation(
                            stg_all[:, b, :], ps[:], act, scale=scale_sb[:, b:b + 1]
                        )
                    nc.scalar.dma_start(agin_re[:, :, :], stg_all[:])
                else:
                    for b in range(NB):
                        ps = small.tile([128, HID], bf16, tag="small")
                        for fc in range(2):
                            nc.tensor.transpose(
                                ps[:, fc * 128:(fc + 1) * 128],
                                src_sb[:, fc, b * 128:(b + 1) * 128],
                                id_bf16[:],
                            )
                        stg = stage.tile([128, HID], featdt, tag="vh")
                        nc.scalar.activation(
                            stg[:], ps[:], act, scale=scale_sb[:, b:b + 1]
                        )
                        nc.scalar.dma_start(agin_re[:, b, :], stg[:])

            epilogue(V1T_sb, agin1_re, AF.Copy, dinvT_sb, "v")

            if RING_REORDER:
                # scalar engine is in-order: these descriptors enter the hw
                # FIFOs only after the agin1 write, keeping the AG1 trigger
                # off the bulk-stream critical path.
                for ck in range(16):
                    nc.scalar.dma_start(
                        adjT_sb[:, ck * 4:(ck + 1) * 4, :],
                        adjT_re[:, ck * 4:(ck + 1) * 4, :],
                    )
                nc.scalar.dma_start(
                    W2_sb[:], W2d[:].rearrange("(p n) f -> p n f", p=128)
                )
                nc.scalar.dma_start(
                    xres_sb[:], xres[:].rearrange("(p n) f -> p n f", p=128)
                )

            def load_featg(agout_re):
                for c in range(CORES):
                    nc.sync.dma_start(
                        featg[:, c * NB:(c + 1) * NB, :], agout_re[:, c, :, :]
                    )

            def aggregate(psum_pair):
                if USE_DR:
                    for jp in range(JB // 2):
                        for fc in range(2):
                            for h in range(2):
                                nc.tensor.matmul(
                                    psum_pair[fc][:, h * 512:(h + 1) * 512],
                                    featg[:, 2 * jp:2 * jp + 2, fc * 128:(fc + 1) * 128],
                                    adjT_sb[:, 2 * jp:2 * jp + 2, h * 512:(h + 1) * 512],
                                    start=(jp == 0),
                                    stop=(jp == JB // 2 - 1),
                                    perf_mode=DR,
                                )
                else:
                    for jb in range(JB):
                        for fc in range(2):
                            for h in range(2):
                                nc.tensor.matmul(
                                    psum_pair[fc][:, h * 512:(h + 1) * 512],
                                    featg[:, jb, fc * 128:(fc + 1) * 128],
                                    adjT_sb[:, jb, h * 512:(h + 1) * 512],
                                    start=(jb == 0),
                                    stop=(jb == JB - 1),
                                )

            nc.gpsimd.collective_compute(
                "AllGather", mybir.AluOpType.bypass, replica_groups=groups,
                ins=[agin1.opt()], outs=[agout1.opt()],
            )
            load_featg(agout1_re)

            # agg1: P1T = featg.T @ adjT  [256, 1024]  (fp8 DoubleRow, K=256/mm)
            p1t = [big.tile([128, R], f32, tag="big", name=f"p1t{i}") for i in range(2)]
            aggregate(p1t)
            for fc in range(2):
                nc.vector.tensor_copy(P1T_sb[:, fc, :], p1t[fc][:])

            # epilogue-2: agin2[i,:] = dinv_i*relu(dinv_i*P1[i,:]) = relu(dinv_i^2*P1)
            epilogue(P1T_sb, agin2_re, AF.Relu, dinv2T_sb, "h")

            nc.gpsimd.collective_compute(
                "AllGather", mybir.AluOpType.bypass, replica_groups=groups,
                ins=[agin2.opt()], outs=[agout2.opt()],
            )
            load_featg(agout2_re)

            # agg2: Q2T = featg.T @ adjT  [256, 1024]
            q2t = [big.tile([128, R], f32, tag="big", name=f"q2t{i}") for i in range(2)]
            aggregate(q2t)
            for fc in range(2):
                nc.vector.tensor_copy(G2T_sb[:, fc, :], q2t[fc][:])

            # Z = Q2 @ W2 per row-block; out = x + alpha*dinv*Z
            for b in range(NB):
                z = big.tile([128, R], f32, tag="big")
                for fc in range(2):
                    for h in range(2):
                        nc.tensor.matmul(
                            z[:, h * 512:(h + 1) * 512],
                            G2T_sb[:, fc, b * 128:(b + 1) * 128],
                            W2_sb[:, fc, h * 512:(h + 1) * 512],
                            start=(fc == 0),
                            stop=(fc == 1),
                        )
                ystg = stage.tile([128, IN_DIM], f32, tag="y")
                nc.scalar.activation(ystg[:], z[:], AF.Copy, scale=adinvT_sb[:, b:b + 1])
                ostg = stage.tile([128, IN_DIM], f32, tag="o")
                if XRES_PRELOAD:
                    nc.vector.tensor_add(ostg[:], ystg[:], xres_sb[:, b, :])
                else:
                    xstg = stage.tile([128, IN_DIM], f32, tag="x")
                    nc.sync.dma_start(xstg[:], xres[b * 128:(b + 1) * 128, :])
                    nc.vector.tensor_add(ostg[:], ystg[:], xstg[:])
                # push on sync so the scalar engine never stalls on the ADD
                nc.sync.dma_start(out[b * 128:(b + 1) * 128, :], ostg[:])

    nc.compile()
    return nc


def _pmaj(a: np.ndarray, nb: int) -> np.ndarray:
    """Permute rows (n p) -> (p n): row p*nb+n of output = row n*128+p of input."""
    r = a.shape[0]
    return np.ascontiguousarray(
        a.reshape(nb, 128, *a.shape[1:]).transpose(1, 0, *range(2, a.ndim + 1))
        .reshape(r, *a.shape[1:])
    )


def kernel(**inputs) -> np.ndarray:
    global LAST_RESULTS
    import ml_dtypes

    x = np.asarray(inputs["x"], dtype=np.float32)
    adj = np.asarray(inputs["adj"], dtype=np.float32)
    W1 = np.asarray(inputs["W1"], dtype=np.float32)
    b1 = np.asarray(inputs["b1"], dtype=np.float32)
    W2 = np.asarray(inputs["W2"], dtype=np.float32)
    b2 = np.asarray(inputs["b2"], dtype=np.float32)
    alpha = float(np.asarray(inputs["alpha"]))
    if np.any(b1) or np.any(b2):
        raise NotImplementedError("nonzero GCN biases not supported")

    bf16 = ml_dtypes.bfloat16
    fp8 = ml_dtypes.float8_e4m3

    adj01 = (adj != 0)
    deg = adj01.sum(axis=1).astype(np.float64) + 1.0   # +1 self-loop
    dinv = (1.0 / np.sqrt(deg)).astype(np.float32)     # [N]

    W1_b = _pmaj(W1.astype(bf16), KB)
    W2_b = _pmaj(W2.astype(bf16), HID // 128)

    in_maps = []
    for c in range(CORES):
        rows = slice(c * R, (c + 1) * R)
        a_c = adj01[rows].T.astype(fp8)               # [N, R], {0,1} exact in fp8
        a_c[c * R + np.arange(R), np.arange(R)] = fp8(1.0)  # self-loops
        dT = np.ascontiguousarray(dinv[rows].reshape(NB, 128).T)  # [128, NB]
        in_maps.append({
            "adjT": _pmaj(a_c, JB),
            "xT": _pmaj(x[rows].T.astype(bf16), KB),
            "xres": (_pmaj(x[rows], NB) if XRES_PRELOAD
                     else np.ascontiguousarray(x[rows])),
            "W1": W1_b,
            "W2": W2_b,
            "dinvT": dT,
            "dinv2T": np.ascontiguousarray(dT * dT),
            "adinvT": np.ascontiguousarray(alpha * dT),
        })

    nc = _build_program(alpha)
    from concourse.bass_utils import run_bass_kernel_spmd
    res = run_bass_kernel_spmd(nc, in_maps, list(range(CORES)))
    LAST_RESULTS = res
    return np.concatenate(
        [np.asarray(res.results[c]["out"], dtype=np.float32) for c in range(CORES)],
        axis=0,
    )


# revision 38
# speedup vs baseline: 1.0092x; 1.0092x over previous
"""Two-layer GCN (PointRefinerGNN) on 8 trn2 cores.

Math: An = norm(adj+I); h = relu(An @ (x@W1)); y = An @ (h@W2); out = x + alpha*y
Sharding: core c owns rows R_c = [c*1024, (c+1)*1024).
Host ships p-major-permuted adjT (fp8 {0,1}), xT/W bf16, x f32 residual,
and precomputed dinv = rsqrt(rowsum+1) transposed tiles.
Device: V1T = (x@W1).T, scale by dinv, AllGather (fp8), aggregate
P1T = feat.T @ adjT with fp8 DoubleRow matmuls, fused relu(dinv^2*P1),
AllGather, aggregate again, Z = Q2@W2, out = x + alpha*dinv*Z.
"""

import numpy as np

N = 8192
IN_DIM = 1024
HID = 256
CORES = 8
R = N // CORES          # 1024 rows per core
NB = R // 128           # 8 row-blocks per core
KB = IN_DIM // 128      # 8 contraction blocks for x@W1
JB = N // 128           # 64 j-blocks for aggregation

LAST_RESULTS = None
USE_FP8_FEAT = True     # featg + AllGather buffers in fp8e4 (else bf16)
USE_DR = True           # DoubleRow perf mode for aggregation matmuls
XRES_PRELOAD = True     # preload all of xres to SBUF (else per-block loads)
AGIN_BATCH = True       # single batched p-major agin write (else per-block)
CHUNK_LOADS = True      # chunked xT/adjT input DMAs, W2 last
RING_REORDER = True     # bulk loads via scalar ring after agin1 write


def _build_program(alpha: float):
    import concourse.bacc as bacc
    import concourse.tile as tile
    import concourse.mybir as mybir
    from concourse import masks

    f32 = mybir.dt.float32
    bf16 = mybir.dt.bfloat16
    fp8 = mybir.dt.float8e4
    featdt = fp8 if USE_FP8_FEAT else bf16
    AF = mybir.ActivationFunctionType
    DR = mybir.MatmulPerfMode.DoubleRow

    nc = bacc.Bacc(
        "TRN2",
        target_bir_lowering=False,
        debug=False,
        enable_asserts=True,
        num_devices=CORES,
    )

    # adjT rows permuted p-major on host: dram row p*JB + n holds source node
    # j = n*128 + p, so each SBUF partition reads one contiguous 64KB run.
    adjT = nc.dram_tensor("adjT", [N, R], fp8, kind="ExternalInput")
    xT = nc.dram_tensor("xT", [IN_DIM, R], bf16, kind="ExternalInput")   # p-major
    xres = nc.dram_tensor("xres", [R, IN_DIM], f32, kind="ExternalInput")
    W1d = nc.dram_tensor("W1", [IN_DIM, HID], bf16, kind="ExternalInput")  # p-major
    W2d = nc.dram_tensor("W2", [HID, IN_DIM], bf16, kind="ExternalInput")  # p-major
    dinvTd = nc.dram_tensor("dinvT", [128, NB], f32, kind="ExternalInput")
    dinv2Td = nc.dram_tensor("dinv2T", [128, NB], f32, kind="ExternalInput")
    adinvTd = nc.dram_tensor("adinvT", [128, NB], f32, kind="ExternalInput")
    out = nc.dram_tensor("out", [R, IN_DIM], f32, kind="ExternalOutput")

    groups = [list(range(CORES))]

    with tile.TileContext(nc) as tc:
        with (
            tc.tile_pool(name="const", bufs=1) as const,
            tc.tile_pool(name="stage", bufs=2) as stage,
            tc.tile_pool(name="big", bufs=3, space="PSUM") as big,
            tc.tile_pool(name="small", bufs=2, space="PSUM") as small,
            tc.tile_pool(name="dram", bufs=1, space="DRAM") as dram,
        ):
            id_bf16 = const.tile([128, 128], bf16)
            masks.make_identity(nc, id_bf16)

            dinvT_sb = const.tile([128, NB], f32)
            nc.sync.dma_start(dinvT_sb[:], dinvTd[:])
            dinv2T_sb = const.tile([128, NB], f32)
            nc.sync.dma_start(dinv2T_sb[:], dinv2Td[:])
            adinvT_sb = const.tile([128, NB], f32)
            nc.sync.dma_start(adinvT_sb[:], adinvTd[:])

            W1_sb = const.tile([128, KB, HID], bf16)
            xT_sb = const.tile([128, KB, R], bf16)
            adjT_sb = const.tile([128, JB, R], fp8)
            W2_sb = const.tile([128, HID // 128, IN_DIM], bf16)
            xT_re = xT[:].rearrange("(p n) f -> p n f", p=128)
            adjT_re = adjT[:].rearrange("(p n) i -> p n i", p=128)

            if RING_REORDER:
                # sync ring carries only what V1T needs; bulk streams go on
                # the scalar ring after the agin1 write (arrival-order FIFOs).
                nc.sync.dma_start(W1_sb[:], W1d[:].rearrange("(p n) f -> p n f", p=128))
                for ck in range(4):
                    nc.sync.dma_start(
                        xT_sb[:, ck * 2:(ck + 1) * 2, :],
                        xT_re[:, ck * 2:(ck + 1) * 2, :],
                    )
            elif CHUNK_LOADS:
                nc.sync.dma_start(W1_sb[:], W1d[:].rearrange("(p n) f -> p n f", p=128))
                for ck in range(4):
                    nc.sync.dma_start(
                        xT_sb[:, ck * 2:(ck + 1) * 2, :],
                        xT_re[:, ck * 2:(ck + 1) * 2, :],
                    )
                for ck in range(16):
                    nc.sync.dma_start(
                        adjT_sb[:, ck * 4:(ck + 1) * 4, :],
                        adjT_re[:, ck * 4:(ck + 1) * 4, :],
                    )
                nc.sync.dma_start(W2_sb[:], W2d[:].rearrange("(p n) f -> p n f", p=128))
            else:
                nc.sync.dma_start(W1_sb[:], W1d[:].rearrange("(p n) f -> p n f", p=128))
                nc.sync.dma_start(W2_sb[:], W2d[:].rearrange("(p n) f -> p n f", p=128))
                nc.sync.dma_start(xT_sb[:], xT_re[:, :, :])
                for ck in range(8):
                    nc.sync.dma_start(
                        adjT_sb[:, ck * 8:(ck + 1) * 8, :],
                        adjT_re[:, ck * 8:(ck + 1) * 8, :],
                    )

            if XRES_PRELOAD:
                xres_sb = const.tile([128, NB, IN_DIM], f32)
                if not RING_REORDER:
                    nc.sync.dma_start(
                        xres_sb[:], xres[:].rearrange("(p n) f -> p n f", p=128)
                    )

            featg = const.tile([128, JB, HID], featdt)

            V1T_sb = const.tile([128, 2, R], bf16)
            P1T_sb = const.tile([128, 2, R], bf16)
            G2T_sb = const.tile([128, 2, R], bf16)

            # AllGather bounce buffers, fp8, p-major rows (row p*NB+b = node
            # b*128+p of the contributing core).
            agin1 = dram.tile([R, HID], featdt)
            agout1 = dram.tile([N, HID], featdt, addr_space="Shared")
            agin2 = dram.tile([R, HID], featdt)
            agout2 = dram.tile([N, HID], featdt, addr_space="Shared")
            agin1_re = agin1.rearrange("(p n) f -> p n f", p=128)
            agin2_re = agin2.rearrange("(p n) f -> p n f", p=128)
            # agout row = c*R + p*NB + b  ->  featg[p, jb=c*NB+b, f]
            agout1_re = agout1.rearrange("(c p n) f -> p c n f", c=CORES, p=128)
            agout2_re = agout2.rearrange("(c p n) f -> p c n f", c=CORES, p=128)

            # V1T = (x_c @ W1).T  [256, 1024]
            v1t = [big.tile([128, R], f32, tag="big", name=f"v1t{i}") for i in range(2)]
            for kb in range(KB):
                for fc in range(2):
                    for h in range(2):
                        nc.tensor.matmul(
                            v1t[fc][:, h * 512:(h + 1) * 512],
                            W1_sb[:, kb, fc * 128:(fc + 1) * 128],
                            xT_sb[:, kb, h * 512:(h + 1) * 512],
                            start=(kb == 0),
                            stop=(kb == KB - 1),
                        )
            for fc in range(2):
                nc.vector.tensor_copy(V1T_sb[:, fc, :], v1t[fc][:])

            # epilogue-1: agin1[i,:] = dinv_i * V1[i,:]  (fp8)
            def epilogue(src_sb, agin_re, act, scale_sb, nm):
                if AGIN_BATCH:
                    stg_all = const.tile([128, NB, HID], featdt, name=f"stg{nm}")
                    for b in range(NB):
                        ps = small.tile([128, HID], bf16, tag="small")
                        for fc in range(2):
                            nc.tensor.transpose(
                                ps[:, fc * 128:(fc + 1) * 128],
                                src_sb[:, fc, b * 128:(b + 1) * 128],
                                id_bf16[:],
                            )
                        nc.scalar.activation(
                            stg_all[:, b, :], ps[:], act, scale=scale_sb[:, b:b + 1]
                        )
                    nc.scalar.dma_start(agin_re[:, :, :], stg_all[:])
                    return stg_all
                else:
                    for b in range(NB):
                        ps = small.tile([128, HID], bf16, tag="small")
                        for fc in range(2):
                            nc.tensor.transpose(
                                ps[:, fc * 128:(fc + 1) * 128],
                                src_sb[:, fc, b * 128:(b + 1) * 128],
                                id_bf16[:],
                            )
                        stg = stage.tile([128, HID], featdt, tag="vh")
                        nc.scalar.activation(
                            stg[:], ps[:], act, scale=scale_sb[:, b:b + 1]
                        )
                        nc.scalar.dma_start(agin_re[:, b, :], stg[:])

            stgv = epilogue(V1T_sb, agin1_re, AF.Copy, dinvT_sb, "v")

            if RING_REORDER:
                # Tiny writes that depend on the epilogue output: the Tile
                # scheduler can't hoist the bulk streams ahead of the agin1
                # write, keeping the AG1 trigger off the bulk critical path
                # (hw DMA FIFOs serve descriptors in arrival order).
                nc.vector.tensor_copy(adjT_sb[:, :, 0:1], stgv[:, NB - 1, 0:JB])
                nc.vector.tensor_copy(W2_sb[:, :, 0:1], stgv[:, NB - 1, 0:2])
                nc.vector.tensor_copy(xres_sb[:, :, 0:1], stgv[:, NB - 1, 0:NB])
                for ck in range(16):
                    nc.scalar.dma_start(
                        adjT_sb[:, ck * 4:(ck + 1) * 4, :],
                        adjT_re[:, ck * 4:(ck + 1) * 4, :],
                    )
                nc.scalar.dma_start(
                    W2_sb[:], W2d[:].rearrange("(p n) f -> p n f", p=128)
                )
                nc.scalar.dma_start(
                    xres_sb[:], xres[:].rearrange("(p n) f -> p n f", p=128)
                )

            def load_featg(agout_re):
                for c in range(CORES):
                    nc.sync.dma_start(
                        featg[:, c * NB:(c + 1) * NB, :], agout_re[:, c, :, :]
                    )

            def aggregate(psum_pair):
                if USE_DR:
                    for jp in range(JB // 2):
                        for fc in range(2):
                            for h in range(2):
                                nc.tensor.matmul(
                                    psum_pair[fc][:, h * 512:(h + 1) * 512],
                                    featg[:, 2 * jp:2 * jp + 2, fc * 128:(fc + 1) * 128],
                                    adjT_sb[:, 2 * jp:2 * jp + 2, h * 512:(h + 1) * 512],
                                    start=(jp == 0),
                                    stop=(jp == JB // 2 - 1),
                                    perf_mode=DR,
                                )
                else:
                    for jb in range(JB):
                        for fc in range(2):
                            for h in range(2):
                                nc.tensor.matmul(
                                    psum_pair[fc][:, h * 512:(h + 1) * 512],
                                    featg[:, jb, fc * 128:(fc + 1) * 128],
                                    adjT_sb[:, jb, h * 512:(h + 1) * 512],
                                    start=(jb == 0),
                                    stop=(jb == JB - 1),
                                )

            nc.gpsimd.collective_compute(
                "AllGather", mybir.AluOpType.bypass, replica_groups=groups,
                ins=[agin1.opt()], outs=[agout1.opt()],
            )
            load_featg(agout1_re)

            # agg1: P1T = featg.T @ adjT  [256, 1024]  (fp8 DoubleRow, K=256/mm)
            p1t = [big.tile([128, R], f32, tag="big", name=f"p1t{i}") for i in range(2)]
            aggregate(p1t)
            for fc in range(2):
                nc.vector.tensor_copy(P1T_sb[:, fc, :], p1t[fc][:])

            # epilogue-2: agin2[i,:] = dinv_i*relu(dinv_i*P1[i,:]) = relu(dinv_i^2*P1)
            epilogue(P1T_sb, agin2_re, AF.Relu, dinv2T_sb, "h")

            nc.gpsimd.collective_compute(
                "AllGather", mybir.AluOpType.bypass, replica_groups=groups,
                ins=[agin2.opt()], outs=[agout2.opt()],
            )
            load_featg(agout2_re)

            # agg2: Q2T = featg.T @ adjT  [256, 1024]
            q2t = [big.tile([128, R], f32, tag="big", name=f"q2t{i}") for i in range(2)]
            aggregate(q2t)
            for fc in range(2):
                nc.vector.tensor_copy(G2T_sb[:, fc, :], q2t[fc][:])

            # Z = Q2 @ W2 per row-block; out = x + alpha*dinv*Z
            for b in range(NB):
                z = big.tile([128, R], f32, tag="big")
                for fc in range(2):
                    for h in range(2):
                        nc.tensor.matmul(
                            z[:, h * 512:(h + 1) * 512],
                            G2T_sb[:, fc, b * 128:(b + 1) * 128],
                            W2_sb[:, fc, h * 512:(h + 1) * 512],
                            start=(fc == 0),
                            stop=(fc == 1),
                        )
                ystg = stage.tile([128, IN_DIM], f32, tag="y")
                nc.scalar.activation(ystg[:], z[:], AF.Copy, scale=adinvT_sb[:, b:b + 1])
                ostg = stage.tile([128, IN_DIM], f32, tag="o")
                if XRES_PRELOAD:
                    nc.vector.tensor_add(ostg[:], ystg[:], xres_sb[:, b, :])
                else:
                    xstg = stage.tile([128, IN_DIM], f32, tag="x")
                    nc.sync.dma_start(xstg[:], xres[b * 128:(b + 1) * 128, :])
                    nc.vector.tensor_add(ostg[:], ystg[:], xstg[:])
                # push on sync so the scalar engine never stalls on the ADD
                nc.sync.dma_start(out[b * 128:(b + 1) * 128, :], ostg[:])

    nc.compile()
    return nc


def _pmaj(a: np.ndarray, nb: int) -> np.ndarray:
    """Permute rows (n p) -> (p n): row p*nb+n of output = row n*128+p of input."""
    r = a.shape[0]
    return np.ascontiguousarray(
        a.reshape(nb, 128, *a.shape[1:]).transpose(1, 0, *range(2, a.ndim + 1))
        .reshape(r, *a.shape[1:])
    )


def kernel(**inputs) -> np.ndarray:
    global LAST_RESULTS
    import ml_dtypes

    x = np.asarray(inputs["x"], dtype=np.float32)
    adj = np.asarray(inputs["adj"], dtype=np.float32)
    W1 = np.asarray(inputs["W1"], dtype=np.float32)
    b1 = np.asarray(inputs["b1"], dtype=np.float32)
    W2 = np.asarray(inputs["W2"], dtype=np.float32)
    b2 = np.asarray(inputs["b2"], dtype=np.float32)
    alpha = float(np.asarray(inputs["alpha"]))
    if np.any(b1) or np.any(b2):
        raise NotImplementedError("nonzero GCN biases not supported")

    bf16 = ml_dtypes.bfloat16
    fp8 = ml_dtypes.float8_e4m3

    adj01 = (adj != 0)
    deg = adj01.sum(axis=1).astype(np.float64) + 1.0   # +1 self-loop
    dinv = (1.0 / np.sqrt(deg)).astype(np.float32)     # [N]

    W1_b = _pmaj(W1.astype(bf16), KB)
    W2_b = _pmaj(W2.astype(bf16), HID // 128)

    in_maps = []
    for c in range(CORES):
        rows = slice(c * R, (c + 1) * R)
        a_c = adj01[rows].T.astype(fp8)               # [N, R], {0,1} exact in fp8
        a_c[c * R + np.arange(R), np.arange(R)] = fp8(1.0)  # self-loops
        dT = np.ascontiguousarray(dinv[rows].reshape(NB, 128).T)  # [128, NB]
        in_maps.append({
            "adjT": _pmaj(a_c, JB),
            "xT": _pmaj(x[rows].T.astype(bf16), KB),
            "xres": (_pmaj(x[rows], NB) if XRES_PRELOAD
                     else np.ascontiguousarray(x[rows])),
            "W1": W1_b,
            "W2": W2_b,
            "dinvT": dT,
            "dinv2T": np.ascontiguousarray(dT * dT),
            "adinvT": np.ascontiguousarray(alpha * dT),
        })

    nc = _build_program(alpha)
    from concourse.bass_utils import run_bass_kernel_spmd
    res = run_bass_kernel_spmd(nc, in_maps, list(range(CORES)))
    LAST_RESULTS = res
    return np.concatenate(
        [np.asarray(res.results[c]["out"], dtype=np.float32) for c in range(CORES)],
        axis=0,
    )
